# revision 4
# baseline (speedup 1.0000x reference)
"""Trainium2 Bass kernel for nn_ComplexNetAttentionBase (complex quantized GQA
attention block).

Distribution: 8 cores = 2 (batch) x 4 (tensor-parallel over kv heads).
Core c handles batch c//4 and kv head r=c%4 (+ its 4 query heads):
  Wq cols [512r:512r+512], Wk/Wv cols [128r:128r+128], Wo rows [512r:512r+512].
Partial outputs (row-parallel Wo) are summed on the host.

Numerics: the reference fake-quantizes activations (per-token int8 absmax)
and weights (sign*mag directions in {+-1, +-i}). Projections therefore run
on integer tensors: activations as round(x*s) stored in bf16 (ints <= 127,
exact) and weight direction matrices in {-1,0,+1} bf16. PSUM accumulates in
fp32, exact for these magnitudes, and the continuous scales (mag, 1/s) are
applied to matmul outputs: per-token scales as per-partition multipliers,
and the softmax score scale grid alpha/(s_i*s_j) via a K=1 outer-product
matmul. Attention itself (scores, softmax, attn@V) runs in fp32 on the PE.
Cross-core reductions (weight magnitude means, per-token o amax) use small
AllGather collectives within each TP group.
"""
import numpy as np

import concourse.bass as bass
import concourse.mybir as mybir
import concourse.tile as tile
from concourse import bass_utils
from concourse.masks import make_identity

F32 = mybir.dt.float32
BF16 = mybir.dt.bfloat16
AX = mybir.AxisListType
OP = mybir.AluOpType
AF = mybir.ActivationFunctionType

B, S, D = 2, 1024, 2048
H, KV, HD = 16, 4, 128
TP = 4                      # tensor-parallel width (kv heads)
QH = H // KV                # q heads per core = 4
NT = S // 128               # 8 token tiles
NKD = D // 128              # 16 contraction chunks over D
MQ = QH * HD                # 512 q cols per core
RC = 12582912.0             # 1.5 * 2^23: fp32 round-to-nearest-even-int magic
N_CORES = 8
GROUPS = [[0, 1, 2, 3], [4, 5, 6, 7]]

# ---------------------------------------------------------------------------
# Walrus in this environment rejects instructions with more than one semaphore
# wait ("Too many sync wait commands"). Split extra waits onto same-engine
# NoOps placed immediately before the instruction (sequencers execute block
# instructions in order, so this is semantics-preserving).
# ---------------------------------------------------------------------------


def _split_excess_waits(nc, limit=1):
    for fn in nc.m.functions:
        for blk in list(fn.blocks):
            newlist = []
            changed = False
            for inst in list(blk.instructions):
                si = inst.sync_info
                waits = list(si.on_wait or []) if si is not None else []
                if len(waits) > limit:
                    changed = True
                    extra, keep = waits[:-limit], waits[-limit:]
                    si.on_wait = keep
                    eng = nc.engines[inst.engine]
                    for i in range(0, len(extra), limit):
                        nop = eng.nop(nofuse=True, hint="wait_split").ins
                        for b2 in fn.blocks:
                            try:
                                b2.instructions.remove(nop)
                                break
                            except ValueError:
                                continue
                        nop.sync_info = mybir.SyncInfo(
                            on_wait=extra[i:i + limit], on_update=[])
                        newlist.append(nop)
                newlist.append(inst)
            if changed:
                blk.instructions = newlist


_tile_patch_done = False


def _apply_tile_patch():
    global _tile_patch_done
    if _tile_patch_done:
        return
    _tile_patch_done = True
    orig_exit = tile.TileContext.__exit__

    def patched_exit(self, *a, **kw):
        ret = orig_exit(self, *a, **kw)
        _split_excess_waits(self.nc)
        return ret

    tile.TileContext.__exit__ = patched_exit


# ---------------------------------------------------------------------------
# kernel build
# ---------------------------------------------------------------------------


def _build():
    nc = bass.Bass("TRN2", target_bir_lowering=False, debug=False,
                   num_devices=N_CORES)

    def din(name, shape):
        return nc.dram_tensor(name, shape, F32, kind="ExternalInput").ap()

    h_re = din("h_re", [S, D])
    h_im = din("h_im", [S, D])
    wq_re, wq_im = din("wq_re", [D, MQ]), din("wq_im", [D, MQ])
    wk_re, wk_im = din("wk_re", [D, HD]), din("wk_im", [D, HD])
    wv_re, wv_im = din("wv_re", [D, HD]), din("wv_im", [D, HD])
    wo_re, wo_im = din("wo_re", [MQ, D]), din("wo_im", [MQ, D])
    cos_t = din("cos_t", [HD, S])          # rope tables, feature-major
    sin_t = din("sin_t", [HD, S])
    tri01 = din("tri01", [128, 128])       # 1 on/below diag else 0
    maskneg = din("maskneg", [128, 128])   # 0 on/below diag else fp32 min
    out = nc.dram_tensor("out", [2, S, D], F32, kind="ExternalOutput").ap()

    def quantize(nc, pool, src, shape, scol, out_bf, tag):
        """out_bf = round(src * scol) in bf16 (matches jnp.round; the clip in
        the reference is a no-op because |x*s| <= 127 by construction)."""
        tmp = pool.tile(list(shape), F32, tag=f"qtmp_{tag}")
        nc.vector.tensor_scalar(out=tmp[:], in0=src, scalar1=scol, scalar2=RC,
                                op0=OP.mult, op1=OP.add)
        nc.vector.tensor_scalar(out=out_bf, in0=tmp[:], scalar1=RC,
                                scalar2=None, op0=OP.subtract)

    with tile.TileContext(nc) as tc, \
            tc.tile_pool(name="consts", bufs=1) as consts, \
            tc.tile_pool(name="misc", bufs=1) as misc, \
            tc.tile_pool(name="dram", bufs=1, space="DRAM") as dram:

        ident_b = consts.tile([128, 128], BF16)
        make_identity(nc, ident_b)
        ident_f = consts.tile([128, 128], F32)
        make_identity(nc, ident_f)
        tri_sb = consts.tile([128, 128], F32)
        nc.sync.dma_start(out=tri_sb[:], in_=tri01[:])
        mneg_sb = consts.tile([128, 128], F32)
        nc.sync.dma_start(out=mneg_sb[:], in_=maskneg[:])
        cos_sb = consts.tile([HD, S], F32)
        nc.sync.dma_start(out=cos_sb[:], in_=cos_t[:])
        sin_sb = consts.tile([HD, S], F32)
        nc.sync.dma_start(out=sin_sb[:], in_=sin_t[:])
        ones_col = consts.tile([128, 1], F32)
        nc.vector.memset(ones_col[:], 1.0)
        ones_row = consts.tile([1, 128], F32)
        nc.vector.memset(ones_row[:], 1.0)

        # small persistent tensors
        s_cols = misc.tile([128, NT], F32)        # s = 127/amax per token
        srecip_cols = misc.tile([128, NT], F32)   # 1/s = amax/127
        mags = misc.tile([128, 4], F32)           # mag_q, mag_k, mag_v, mag_o
        srow = misc.tile([1, S], F32)             # 1/s as a row
        arow = misc.tile([1, S], F32)             # alpha * (1/s) as a row
        so_cols = misc.tile([128, NT], F32)
        oscale_cols = misc.tile([128, NT], F32)

        lam = misc.tile([128, NT], F32)           # local |o| amax per token
        o_red = dram.tile([NT, 128, MQ], F32)     # attention out (tok-major)
        o_imd = dram.tile([NT, 128, MQ], F32)

        with tc.tile_pool(name="qkv", bufs=1) as qkv:
            qT_re = qkv.tile([128, QH, S], F32)      # integer q, feature-major
            qT_im = qkv.tile([128, QH, S], F32)
            kT_re = qkv.tile([128, S], F32)
            kT_im = qkv.tile([128, S], F32)
            vs_re = qkv.tile([128, NT, HD], F32)     # tok-major, pre-scaled
            vs_im = qkv.tile([128, NT, HD], F32)

            with tc.tile_pool(name="qt", bufs=1) as qtp:
                qt_re = qtp.tile([128, NKD, S], BF16)  # quantized hidden^T
                qt_im = qtp.tile([128, NKD, S], BF16)

                # ----------------------------------------------------------
                # Phase A: act_quant(hidden) + transpose to feature-major
                # ----------------------------------------------------------
                with tc.tile_pool(name="pha", bufs=2) as pha, \
                        tc.tile_pool(name="pha_ps", bufs=4,
                                     space="PSUM") as pha_ps:
                    for t in range(NT):
                        ts_ = slice(t * 128, (t + 1) * 128)
                        hr = pha.tile([128, D], F32, tag="hr")
                        hi = pha.tile([128, D], F32, tag="hi")
                        nc.sync.dma_start(out=hr[:], in_=h_re[ts_, :])
                        nc.sync.dma_start(out=hi[:], in_=h_im[ts_, :])
                        amr = pha.tile([128, 1], F32, tag="amr")
                        ami = pha.tile([128, 1], F32, tag="ami")
                        nc.vector.tensor_reduce(
                            out=amr[:], in_=hr[:], axis=AX.X, op=OP.max,
                            apply_absolute_value=True)
                        nc.vector.tensor_reduce(
                            out=ami[:], in_=hi[:], axis=AX.X, op=OP.max,
                            apply_absolute_value=True)
                        am = pha.tile([128, 1], F32, tag="am")
                        nc.vector.tensor_max(am[:], amr[:], ami[:])
                        nc.vector.tensor_scalar_max(out=am[:], in0=am[:],
                                                    scalar1=1e-5)
                        rec = pha.tile([128, 1], F32, tag="rec")
                        nc.vector.reciprocal(out=rec[:], in_=am[:])
                        nc.vector.tensor_scalar_mul(
                            out=s_cols[:, t:t + 1], in0=rec[:], scalar1=127.0)
                        nc.vector.tensor_scalar_mul(
                            out=srecip_cols[:, t:t + 1], in0=am[:],
                            scalar1=1.0 / 127.0)
                        qr = pha.tile([128, D], BF16, tag="qr")
                        qi = pha.tile([128, D], BF16, tag="qi")
                        quantize(nc, pha, hr[:], [128, D],
                                 s_cols[:, t:t + 1], qr[:], "a")
                        quantize(nc, pha, hi[:], [128, D],
                                 s_cols[:, t:t + 1], qi[:], "a")
                        for k in range(NKD):
                            ks = slice(k * 128, (k + 1) * 128)
                            p1 = pha_ps.tile([128, 128], BF16, tag="tp")
                            nc.tensor.transpose(p1[:], qr[:, ks], ident_b[:])
                            nc.vector.tensor_copy(out=qt_re[:, k, ts_],
                                                  in_=p1[:])
                            p2 = pha_ps.tile([128, 128], BF16, tag="tp")
                            nc.tensor.transpose(p2[:], qi[:, ks], ident_b[:])
                            nc.vector.tensor_copy(out=qt_im[:, k, ts_],
                                                  in_=p2[:])

                    # 1/s as a row [1, S] (via PE transpose + DRAM bounce)
                    rr_ps = pha_ps.tile([128, 128], F32, tag="rrps")
                    nc.tensor.transpose(rr_ps[0:NT, :], srecip_cols[:],
                                        ident_f[:])
                    rr_sb = pha.tile([NT, 128], F32, tag="rrsb")
                    nc.vector.tensor_copy(out=rr_sb[:], in_=rr_ps[0:NT, :])
                    srow_d = dram.tile([NT, 128], F32)
                    nc.sync.dma_start(out=srow_d[:], in_=rr_sb[:])
                    nc.sync.dma_start(
                        out=srow[:1, :],
                        in_=srow_d[:].rearrange("t p -> (t p)"))

                # ----------------------------------------------------------
                # Phase B: weight dirs + magnitude partials + AllGather
                # ----------------------------------------------------------
                with tc.tile_pool(name="wdirs", bufs=1) as wdirs:
                    dr_q = wdirs.tile([128, NKD, MQ], BF16)
                    di_q = wdirs.tile([128, NKD, MQ], BF16)
                    dr_k = wdirs.tile([128, NKD, HD], BF16)
                    di_k = wdirs.tile([128, NKD, HD], BF16)
                    dr_v = wdirs.tile([128, NKD, HD], BF16)
                    di_v = wdirs.tile([128, NKD, HD], BF16)

                    with tc.tile_pool(name="phb", bufs=1) as phb, \
                            tc.tile_pool(name="phb_ps", bufs=2,
                                         space="PSUM") as phb_ps:
                        magcols = phb.tile([128, 4], F32, tag="magcols")

                        def wdir_chunk(wr_ap, wi_ap, nk, m, drt, dit, slot,
                                       first):
                            wr = phb.tile([128, nk, m], F32, tag="wr")
                            wi = phb.tile([128, nk, m], F32, tag="wi")
                            nc.sync.dma_start(out=wr[:], in_=wr_ap)
                            nc.sync.dma_start(out=wi[:], in_=wi_ap)
                            m1 = phb.tile([128, nk, m], F32, tag="m1")
                            m2 = phb.tile([128, nk, m], F32, tag="m2")
                            nc.vector.tensor_mul(m1[:], wr[:], wr[:])
                            nc.vector.tensor_mul(m2[:], wi[:], wi[:])
                            mask = None
                            if drt is not None:
                                mask = phb.tile([128, nk, m], F32, tag="mask")
                                nc.vector.tensor_tensor(
                                    out=mask[:], in0=m1[:], in1=m2[:],
                                    op=OP.is_ge)
                            nc.vector.tensor_add(m1[:], m1[:], m2[:])
                            csum = phb.tile([128, 1], F32, tag="csum")
                            nc.scalar.activation(out=m1[:], in_=m1[:],
                                                 func=AF.Sqrt,
                                                 accum_out=csum[:])
                            if first:
                                nc.vector.tensor_copy(
                                    out=magcols[:, slot:slot + 1], in_=csum[:])
                            else:
                                nc.vector.tensor_add(
                                    magcols[:, slot:slot + 1],
                                    magcols[:, slot:slot + 1], csum[:])
                            if drt is not None:
                                sg = phb.tile([128, nk, m], F32, tag="sg")
                                nc.scalar.activation(out=sg[:], in_=wr[:],
                                                     func=AF.Sign)
                                nc.vector.tensor_mul(drt, sg[:], mask[:])
                                nc.vector.tensor_scalar(
                                    out=mask[:], in0=mask[:], scalar1=-1.0,
                                    scalar2=1.0, op0=OP.mult, op1=OP.add)
                                sg2 = phb.tile([128, nk, m], F32, tag="sg")
                                nc.scalar.activation(out=sg2[:], in_=wi[:],
                                                     func=AF.Sign)
                                nc.vector.tensor_mul(dit, sg2[:], mask[:])

                        wqr = wq_re.rearrange("(k p) m -> p k m", p=128)
                        wqi = wq_im.rearrange("(k p) m -> p k m", p=128)
                        for i in range(8):
                            ks = slice(i * 2, (i + 1) * 2)
                            wdir_chunk(wqr[:, ks, :], wqi[:, ks, :], 2, MQ,
                                       dr_q[:, ks, :], di_q[:, ks, :], 0,
                                       i == 0)
                        wkr = wk_re.rearrange("(k p) m -> p k m", p=128)
                        wki = wk_im.rearrange("(k p) m -> p k m", p=128)
                        wvr = wv_re.rearrange("(k p) m -> p k m", p=128)
                        wvi = wv_im.rearrange("(k p) m -> p k m", p=128)
                        for i in range(2):
                            ks = slice(i * 8, (i + 1) * 8)
                            wdir_chunk(wkr[:, ks, :], wki[:, ks, :], 8, HD,
                                       dr_k[:, ks, :], di_k[:, ks, :], 1,
                                       i == 0)
                            wdir_chunk(wvr[:, ks, :], wvi[:, ks, :], 8, HD,
                                       dr_v[:, ks, :], di_v[:, ks, :], 2,
                                       i == 0)
                        wor = wo_re.rearrange("(k p) m -> p k m", p=128)
                        woi = wo_im.rearrange("(k p) m -> p k m", p=128)
                        for i in range(4):
                            for half in range(2):
                                ms_ = slice(half * 1024, (half + 1) * 1024)
                                wdir_chunk(wor[:, i:i + 1, ms_],
                                           woi[:, i:i + 1, ms_],
                                           1, 1024, None, None, 3,
                                           i == 0 and half == 0)

                        # partition-sum -> [1,4] -> AllGather -> broadcast
                        mg_ps = phb_ps.tile([1, 4], F32, tag="mgps")
                        nc.tensor.matmul(mg_ps[:], ones_col[:], magcols[:],
                                         start=True, stop=True)
                        mg_sb = phb.tile([1, 4], F32, tag="mgsb")
                        nc.vector.tensor_copy(out=mg_sb[:], in_=mg_ps[:])
                        ag1_in = dram.tile([1, 4], F32)
                        ag1_out = dram.tile([TP, 4], F32)
                        nc.sync.dma_start(out=ag1_in[:], in_=mg_sb[:])
                        nc.gpsimd.collective_compute(
                            "AllGather", OP.bypass, replica_groups=GROUPS,
                            ins=[ag1_in.opt()], outs=[ag1_out.opt()])
                        row16 = phb.tile([1, 16], F32, tag="row16")
                        nc.sync.dma_start(
                            out=row16[:1, :],
                            in_=ag1_out[:].rearrange("r m -> (r m)"))
                        bc_ps = phb_ps.tile([128, 16], F32, tag="bcps")
                        nc.tensor.matmul(bc_ps[:], ones_row[:1, :],
                                         row16[:1, :], start=True, stop=True)
                        mg_all = phb.tile([128, 16], F32, tag="mgall")
                        nc.vector.tensor_copy(out=mg_all[:], in_=bc_ps[:])
                        mg_sum = phb.tile([128, 4], F32, tag="mgsum")
                        nc.vector.tensor_reduce(
                            out=mg_sum[:],
                            in_=mg_all[:].rearrange("p (r m) -> p m r", m=4),
                            axis=AX.X, op=OP.add)
                        for i, numel in enumerate(
                                [D * D, D * KV * HD, D * KV * HD, D * D]):
                            nc.vector.tensor_scalar_mul(
                                out=mags[:, i:i + 1], in0=mg_sum[:, i:i + 1],
                                scalar1=1.0 / float(numel))
                        alpha = phb.tile([128, 1], F32, tag="alpha")
                        nc.vector.tensor_mul(alpha[:], mags[:, 0:1],
                                             mags[:, 1:2])
                        nc.vector.tensor_scalar_mul(
                            out=alpha[:], in0=alpha[:],
                            scalar1=float(1.0 / np.sqrt(np.float32(HD))))
                        nc.vector.tensor_scalar(
                            out=arow[:], in0=srow[:],
                            scalar1=alpha[0:1, 0:1], scalar2=None,
                            op0=OP.mult)

                    # ------------------------------------------------------
                    # Phase C: QKV projections (integer bf16 matmuls) + RoPE
                    # ------------------------------------------------------
                    with tc.tile_pool(name="phc", bufs=2) as phc, \
                            tc.tile_pool(name="phc_ps", bufs=4,
                                         space="PSUM") as phc_ps:

                        def proj_fm(dirA, dirB, f, n):
                            """psum[feat 128, tok 512] = sum_k dirA[:,k,f]^T @
                            qt_re[:,k,n] + dirB[:,k,f]^T @ qt_im[:,k,n]"""
                            ps = phc_ps.tile([128, 512], F32, tag="proj")
                            t0 = n * 512
                            fs = slice(f * 128, (f + 1) * 128)
                            for k in range(NKD):
                                nc.tensor.matmul(
                                    ps[:], dirA[:, k, fs],
                                    qt_re[:, k, t0:t0 + 512],
                                    start=(k == 0), stop=False)
                            for k in range(NKD):
                                nc.tensor.matmul(
                                    ps[:], dirB[:, k, fs],
                                    qt_im[:, k, t0:t0 + 512],
                                    start=False, stop=(k == NKD - 1))
                            return ps

                        def proj_tm(dirA, dirB, t):
                            """psum[tok 128, HD] = sum_k qt_re[:,k,t]^T@dirA +
                            qt_im[:,k,t]^T@dirB (token-major, for V)."""
                            ps = phc_ps.tile([128, HD], F32, tag="vproj")
                            ts_ = slice(t * 128, (t + 1) * 128)
                            for k in range(NKD):
                                nc.tensor.matmul(ps[:], qt_re[:, k, ts_],
                                                 dirA[:, k, :],
                                                 start=(k == 0), stop=False)
                            for k in range(NKD):
                                nc.tensor.matmul(ps[:], qt_im[:, k, ts_],
                                                 dirB[:, k, :], start=False,
                                                 stop=(k == NKD - 1))
                            return ps

                        def vscale(t):
                            vsc = phc.tile([128, 1], F32, tag="vsc")
                            nc.vector.tensor_scalar(
                                out=vsc[:], in0=srecip_cols[:, t:t + 1],
                                scalar1=mags[:, 2:3], scalar2=None,
                                op0=OP.mult)
                            return vsc

                        # imaginary outputs first (use +qt_im)
                        for f in range(QH):
                            for n in range(2):
                                ps = proj_fm(di_q, dr_q, f, n)
                                nc.vector.tensor_copy(
                                    out=qT_im[:, f, n * 512:(n + 1) * 512],
                                    in_=ps[:])
                        for n in range(2):
                            ps = proj_fm(di_k, dr_k, 0, n)
                            nc.vector.tensor_copy(
                                out=kT_im[:, n * 512:(n + 1) * 512], in_=ps[:])
                        for t in range(NT):
                            ps = proj_tm(di_v, dr_v, t)
                            vsc = vscale(t)
                            nc.vector.tensor_scalar(
                                out=vs_im[:, t, :], in0=ps[:], scalar1=vsc[:],
                                scalar2=None, op0=OP.mult)

                        # negate qt_im in place (ints, exact), then real parts
                        nc.vector.tensor_scalar(
                            out=qt_im[:], in0=qt_im[:], scalar1=-1.0,
                            scalar2=None, op0=OP.mult)

                        for f in range(QH):
                            for n in range(2):
                                ps = proj_fm(dr_q, di_q, f, n)
                                nc.vector.tensor_copy(
                                    out=qT_re[:, f, n * 512:(n + 1) * 512],
                                    in_=ps[:])
                        for n in range(2):
                            ps = proj_fm(dr_k, di_k, 0, n)
                            nc.vector.tensor_copy(
                                out=kT_re[:, n * 512:(n + 1) * 512], in_=ps[:])
                        for t in range(NT):
                            ps = proj_tm(dr_v, di_v, t)
                            vsc = vscale(t)
                            nc.vector.tensor_scalar(
                                out=vs_re[:, t, :], in0=ps[:], scalar1=vsc[:],
                                scalar2=None, op0=OP.mult)

                        # RoPE on real parts (feature-major). rotate_half is
                        # a partition swap; two-SB-input ops need equal base
                        # partitions, so stage the rotated (sign-folded) copy
                        # first with single-input ops.
                        def rope_apply(sub):
                            a = phc.tile([128, S], F32, tag="ropea")
                            nc.vector.tensor_mul(a[:], sub(0, 128), cos_sb[:])
                            bt = phc.tile([128, S], F32, tag="ropeb")
                            nc.vector.tensor_scalar(
                                out=bt[0:64, :], in0=sub(64, 128),
                                scalar1=-1.0, scalar2=None, op0=OP.mult)
                            nc.vector.tensor_copy(out=bt[64:128, :],
                                                  in_=sub(0, 64))
                            nc.vector.tensor_mul(bt[:], bt[:], sin_sb[:])
                            nc.vector.tensor_add(sub(0, 128), a[:], bt[:])

                        for f in range(QH):
                            rope_apply(lambda lo, hi, f=f: qT_re[lo:hi, f, :])
                        rope_apply(lambda lo, hi: kT_re[lo:hi, :])

            # --------------------------------------------------------------
            # Phase D: attention (fp32, causal)
            # --------------------------------------------------------------
            with tc.tile_pool(name="phd", bufs=2) as phd, \
                    tc.tile_pool(name="ps_bg", bufs=1, space="PSUM") as ps_bg, \
                    tc.tile_pool(name="ps_sc", bufs=1, space="PSUM") as ps_sc, \
                    tc.tile_pool(name="ps_at", bufs=2, space="PSUM") as ps_at, \
                    tc.tile_pool(name="ps_av", bufs=1, space="PSUM") as ps_av:
                for t in range(NT):
                    klen = (t + 1) * 128
                    qs = slice(t * 128, (t + 1) * 128)
                    # scale grid B[i,j] = alpha/(s_i*s_j), causal-masked
                    bg_ps = ps_bg.tile([128, 1024], F32, tag="bg")
                    for c in range(0, klen, 512):
                        ce = min(c + 512, klen)
                        nc.tensor.matmul(bg_ps[:, c:ce], arow[:1, qs],
                                         srow[:1, c:ce], start=True, stop=True)
                    bg = phd.tile([128, 1024], F32, tag="bgsb")
                    nc.vector.tensor_copy(out=bg[:, :klen], in_=bg_ps[:, :klen])
                    nc.vector.tensor_mul(bg[:, t * 128:klen],
                                         bg[:, t * 128:klen], tri_sb[:])
                    for h in range(QH):
                        sc_ps = ps_sc.tile([128, 1024], F32, tag="scores")
                        for c in range(0, klen, 512):
                            ce = min(c + 512, klen)
                            nc.tensor.matmul(sc_ps[:, c:ce], qT_re[:, h, qs],
                                             kT_re[:, c:ce], start=True,
                                             stop=False)
                            nc.tensor.matmul(sc_ps[:, c:ce], qT_im[:, h, qs],
                                             kT_im[:, c:ce], start=False,
                                             stop=True)
                        sv = phd.tile([128, 1024], F32, tag="sv")
                        nc.vector.tensor_mul(sv[:, :klen], sc_ps[:, :klen],
                                             bg[:, :klen])
                        nc.vector.tensor_add(sv[:, t * 128:klen],
                                             sv[:, t * 128:klen], mneg_sb[:])
                        negm = phd.tile([128, 1], F32, tag="negm")
                        nc.vector.tensor_reduce(out=negm[:], in_=sv[:, :klen],
                                                axis=AX.X, op=OP.max,
                                                negate=True)
                        pexp = phd.tile([128, 1024], F32, tag="pexp")
                        esum = phd.tile([128, 1], F32, tag="esum")
                        nc.scalar.activation(out=pexp[:, :klen],
                                             in_=sv[:, :klen], func=AF.Exp,
                                             bias=negm[:], scale=1.0,
                                             accum_out=esum[:])
                        at_sb = phd.tile([128, 1024], F32, tag="atsb")
                        for kc in range(t + 1):
                            kcs = slice(kc * 128, (kc + 1) * 128)
                            tp = ps_at.tile([128, 128], F32, tag="attnT")
                            nc.tensor.transpose(tp[:], pexp[:, kcs], ident_f[:])
                            nc.vector.tensor_copy(out=at_sb[:, kcs], in_=tp[:])
                        ore_ps = ps_av.tile([128, HD], F32, tag="avre")
                        oim_ps = ps_av.tile([128, HD], F32, tag="avim")
                        for kc in range(t + 1):
                            kcs = slice(kc * 128, (kc + 1) * 128)
                            nc.tensor.matmul(ore_ps[:], at_sb[:, kcs],
                                             vs_re[:, kc, :], start=(kc == 0),
                                             stop=(kc == t))
                            nc.tensor.matmul(oim_ps[:], at_sb[:, kcs],
                                             vs_im[:, kc, :], start=(kc == 0),
                                             stop=(kc == t))
                        rsum = phd.tile([128, 1], F32, tag="rsum")
                        nc.vector.reciprocal(out=rsum[:], in_=esum[:])
                        hs = slice(h * HD, (h + 1) * HD)
                        osb_re = phd.tile([128, HD], F32, tag="osbre")
                        osb_im = phd.tile([128, HD], F32, tag="osbim")
                        nc.vector.tensor_scalar(
                            out=osb_re[:], in0=ore_ps[:], scalar1=rsum[:],
                            scalar2=None, op0=OP.mult)
                        nc.vector.tensor_scalar(
                            out=osb_im[:], in0=oim_ps[:], scalar1=rsum[:],
                            scalar2=None, op0=OP.mult)
                        nc.sync.dma_start(out=o_red[t, :, hs], in_=osb_re[:])
                        nc.sync.dma_start(out=o_imd[t, :, hs], in_=osb_im[:])
                        am1 = phd.tile([128, 1], F32, tag="dam1")
                        am2 = phd.tile([128, 1], F32, tag="dam2")
                        nc.vector.tensor_reduce(
                            out=am1[:], in_=osb_re[:], axis=AX.X, op=OP.max,
                            apply_absolute_value=True)
                        nc.vector.tensor_reduce(
                            out=am2[:], in_=osb_im[:], axis=AX.X, op=OP.max,
                            apply_absolute_value=True)
                        nc.vector.tensor_max(am1[:], am1[:], am2[:])
                        if h == 0:
                            nc.vector.tensor_copy(out=lam[:, t:t + 1],
                                                  in_=am1[:])
                        else:
                            nc.vector.tensor_max(lam[:, t:t + 1],
                                                 lam[:, t:t + 1], am1[:])

        # ------------------------------------------------------------------
        # Phase E: o act_quant (amax AllGather) + output projection
        # ------------------------------------------------------------------
        with tc.tile_pool(name="phe_w", bufs=1) as phe_w, \
                tc.tile_pool(name="phe", bufs=1) as phe, \
                tc.tile_pool(name="phe_ps", bufs=2, space="PSUM") as phe_ps:
            ag2_in = dram.tile([128, NT], F32)
            ag2_out = dram.tile([TP * 128, NT], F32)
            nc.sync.dma_start(out=ag2_in[:], in_=lam[:])
            nc.gpsimd.collective_compute(
                "AllGather", OP.bypass, replica_groups=GROUPS,
                ins=[ag2_in.opt()], outs=[ag2_out.opt()])
            gam = phe.tile([128, NT, TP], F32, tag="gam")
            nc.sync.dma_start(
                out=gam[:],
                in_=ag2_out[:].rearrange("(r p) t -> p t r", p=128))
            amg = phe.tile([128, NT], F32, tag="amg")
            nc.vector.tensor_reduce(out=amg[:], in_=gam[:], axis=AX.X,
                                    op=OP.max)
            nc.vector.tensor_scalar_max(out=amg[:], in0=amg[:], scalar1=1e-5)
            orec = phe.tile([128, NT], F32, tag="orec")
            nc.vector.reciprocal(out=orec[:], in_=amg[:])
            nc.vector.tensor_scalar_mul(out=so_cols[:], in0=orec[:],
                                        scalar1=127.0)
            nc.vector.tensor_scalar(out=oscale_cols[:], in0=amg[:],
                                    scalar1=1.0 / 127.0, scalar2=None,
                                    op0=OP.mult)
            nc.vector.tensor_scalar(out=oscale_cols[:], in0=oscale_cols[:],
                                    scalar1=mags[:, 3:4], scalar2=None,
                                    op0=OP.mult)

            # quantize o and transpose to feature-major (or_t / oi_t / -oi_t)
            or_t = phe_w.tile([128, QH, S], BF16)
            oi_t = phe_w.tile([128, QH, S], BF16)
            oineg_t = phe_w.tile([128, QH, S], BF16)
            for t in range(NT):
                ts_ = slice(t * 128, (t + 1) * 128)
                olr = phe.tile([128, MQ], F32, tag="olr")
                oli = phe.tile([128, MQ], F32, tag="oli")
                nc.sync.dma_start(out=olr[:], in_=o_red[t, :, :])
                nc.sync.dma_start(out=oli[:], in_=o_imd[t, :, :])
                qr = phe.tile([128, MQ], BF16, tag="oqr")
                qi = phe.tile([128, MQ], BF16, tag="oqi")
                quantize(nc, phe, olr[:], [128, MQ],
                         so_cols[:, t:t + 1], qr[:], "e")
                quantize(nc, phe, oli[:], [128, MQ],
                         so_cols[:, t:t + 1], qi[:], "e")
                for f in range(QH):
                    fs = slice(f * 128, (f + 1) * 128)
                    p1 = phe_ps.tile([128, 128], BF16, tag="otp")
                    nc.tensor.transpose(p1[:], qr[:, fs], ident_b[:])
                    nc.vector.tensor_copy(out=or_t[:, f, ts_], in_=p1[:])
                    p2 = phe_ps.tile([128, 128], BF16, tag="otp")
                    nc.tensor.transpose(p2[:], qi[:, fs], ident_b[:])
                    nc.vector.tensor_copy(out=oi_t[:, f, ts_], in_=p2[:])
                    nc.vector.tensor_scalar(
                        out=oineg_t[:, f, ts_], in0=p2[:], scalar1=-1.0,
                        scalar2=None, op0=OP.mult)

            # wo direction matrices
            dr_o = phe_w.tile([128, QH, D], BF16)
            di_o = phe_w.tile([128, QH, D], BF16)
            wor = wo_re.rearrange("(k p) m -> p k m", p=128)
            woi = wo_im.rearrange("(k p) m -> p k m", p=128)
            for i in range(4):
                wr = phe.tile([128, 1, D], F32, tag="ewr")
                wi = phe.tile([128, 1, D], F32, tag="ewi")
                nc.sync.dma_start(out=wr[:], in_=wor[:, i:i + 1, :])
                nc.sync.dma_start(out=wi[:], in_=woi[:, i:i + 1, :])
                m1 = phe.tile([128, 1, D], F32, tag="em1")
                m2 = phe.tile([128, 1, D], F32, tag="em2")
                nc.vector.tensor_mul(m1[:], wr[:], wr[:])
                nc.vector.tensor_mul(m2[:], wi[:], wi[:])
                mask = phe.tile([128, 1, D], F32, tag="emask")
                nc.vector.tensor_tensor(out=mask[:], in0=m1[:], in1=m2[:],
                                        op=OP.is_ge)
                sg = phe.tile([128, 1, D], F32, tag="esg")
                nc.scalar.activation(out=sg[:], in_=wr[:], func=AF.Sign)
                nc.vector.tensor_mul(dr_o[:, i:i + 1, :], sg[:], mask[:])
                nc.vector.tensor_scalar(out=mask[:], in0=mask[:], scalar1=-1.0,
                                        scalar2=1.0, op0=OP.mult, op1=OP.add)
                sg2 = phe.tile([128, 1, D], F32, tag="esg")
                nc.scalar.activation(out=sg2[:], in_=wi[:], func=AF.Sign)
                nc.vector.tensor_mul(di_o[:, i:i + 1, :], sg2[:], mask[:])

            # output projection: out[tok, D] partial sums
            for t in range(NT):
                ts_ = slice(t * 128, (t + 1) * 128)
                for oc in range(4):
                    ocs = slice(oc * 512, (oc + 1) * 512)
                    ps_re = phe_ps.tile([128, 512], F32, tag="opre")
                    ps_im = phe_ps.tile([128, 512], F32, tag="opim")
                    for f in range(QH):
                        nc.tensor.matmul(ps_re[:], or_t[:, f, ts_],
                                         dr_o[:, f, ocs], start=(f == 0),
                                         stop=False)
                        nc.tensor.matmul(ps_im[:], or_t[:, f, ts_],
                                         di_o[:, f, ocs], start=(f == 0),
                                         stop=False)
                    for f in range(QH):
                        nc.tensor.matmul(ps_re[:], oineg_t[:, f, ts_],
                                         di_o[:, f, ocs], start=False,
                                         stop=(f == QH - 1))
                        nc.tensor.matmul(ps_im[:], oi_t[:, f, ts_],
                                         dr_o[:, f, ocs], start=False,
                                         stop=(f == QH - 1))
                    fin_re = phe.tile([128, 512], F32, tag="finre")
                    fin_im = phe.tile([128, 512], F32, tag="finim")
                    nc.vector.tensor_scalar(out=fin_re[:], in0=ps_re[:],
                                            scalar1=oscale_cols[:, t:t + 1],
                                            scalar2=None, op0=OP.mult)
                    nc.vector.tensor_scalar(out=fin_im[:], in0=ps_im[:],
                                            scalar1=oscale_cols[:, t:t + 1],
                                            scalar2=None, op0=OP.mult)
                    nc.sync.dma_start(out=out[0, ts_, ocs], in_=fin_re[:])
                    nc.sync.dma_start(out=out[1, ts_, ocs], in_=fin_im[:])

    return nc


# ---------------------------------------------------------------------------
# host wrapper
# ---------------------------------------------------------------------------

_NC_CACHE = {}


def _rope_tables_np():
    inv_freq = (1.0 / (np.float32(10000.0) **
                       (np.arange(0, HD, 2, dtype=np.float32) /
                        np.float32(HD)))).astype(np.float32)
    pos = np.arange(S, dtype=np.float32)
    freqs = (pos[:, None] * inv_freq[None, :]).astype(np.float32)
    emb = np.concatenate([freqs, freqs], axis=-1)
    return np.cos(emb).astype(np.float32), np.sin(emb).astype(np.float32)


def _make_in_maps(inputs):
    cos, sin = _rope_tables_np()
    cos_t = np.ascontiguousarray(cos.T)
    sin_t = np.ascontiguousarray(sin.T)
    ii, jj = np.indices((128, 128))
    tri01 = (jj <= ii).astype(np.float32)
    maskneg = np.where(jj > ii, np.finfo(np.float32).min,
                       np.float32(0.0)).astype(np.float32)

    in_maps = []
    for c in range(N_CORES):
        b, r = c // TP, c % TP
        in_maps.append({
            "h_re": np.ascontiguousarray(inputs["hidden_real"][b]),
            "h_im": np.ascontiguousarray(inputs["hidden_imag"][b]),
            "wq_re": np.ascontiguousarray(inputs["wq_re"][:, r * MQ:(r + 1) * MQ]),
            "wq_im": np.ascontiguousarray(inputs["wq_im"][:, r * MQ:(r + 1) * MQ]),
            "wk_re": np.ascontiguousarray(inputs["wk_re"][:, r * HD:(r + 1) * HD]),
            "wk_im": np.ascontiguousarray(inputs["wk_im"][:, r * HD:(r + 1) * HD]),
            "wv_re": np.ascontiguousarray(inputs["wv_re"][:, r * HD:(r + 1) * HD]),
            "wv_im": np.ascontiguousarray(inputs["wv_im"][:, r * HD:(r + 1) * HD]),
            "wo_re": np.ascontiguousarray(inputs["wo_re"][r * MQ:(r + 1) * MQ, :]),
            "wo_im": np.ascontiguousarray(inputs["wo_im"][r * MQ:(r + 1) * MQ, :]),
            "cos_t": cos_t, "sin_t": sin_t,
            "tri01": tri01, "maskneg": maskneg,
        })
    return in_maps


def kernel(**inputs):
    _apply_tile_patch()
    if "nc" not in _NC_CACHE:
        _NC_CACHE["nc"] = _build()
    nc = _NC_CACHE["nc"]
    in_maps = _make_in_maps(inputs)
    res = bass_utils.run_bass_kernel_spmd(nc, in_maps, list(range(N_CORES)))
    full = np.zeros((2, B, S, D), dtype=np.float32)
    for b in range(B):
        for r in range(TP):
            full[:, b] += res.results[b * TP + r]["out"]
    return full
